# revision 36
# baseline (speedup 1.0000x reference)
"""Fused multi-head attention forward (B=2, S=2048, SIZE=1024, H=16) on 8
Trainium2 NeuronCores.

Sharding: 2-way data parallel over batch x 4-way tensor parallel over heads
(Megatron style). Each core computes 4 heads of one batch element end-to-end
(QKV projections for its 256-dim slice, attention, and a partial output
projection); the host sums the 4 partials per batch and adds the output
bias. The value-projection bias drops out of attention algebraically
(softmax rows sum to 1), so the host folds `bv @ Wo.T` into that same
constant row.

Single software-pipelined schedule built around the scalar-engine exp
stream (the hard floor: S*S*H_LOC exps/core at 1 elem/lane/cycle). The
attention kt-loop keeps ACT busy with [128,1024] exp instructions while
the PE fills its idle cycles from a unit queue carrying the QKV
projections for the *next* head pair, the output projection for
*previous* q-ranges, and the softmax-denominator reduction:

  qhT/khT [dim, s]  <- qT/kT rows as rhs, WqT/WkT as lhsT (no transposes)
  vh      [s, dim]  <- vT rows as lhsT, WvT as rhs
  scoresT [k, q]    <- khT as lhsT (c=64); two heads packed in PE row
                       groups 0-63 / 64-127 run as concurrent streams
  exp     ACT PSUM->SBUF bf16, one [128,1024] instruction per head
  ctxT    [dim, q]  <- vh as lhsT, exp(scoresT) as rhs; two heads packed
                       in PE col groups (output partitions 0-63 / 64-127)
  denom:  exp tiles accumulated on DVE into 4 interleaved bf16 tiles,
          then reduced over partitions AND broadcast in one shot by
          ones[128,64] matmuls accumulated in PSUM; reciprocal +
          normalize on DVE read the ctx PSUM banks directly
  out     [s, o]    <- ctxT as lhsT, WoT as rhs (c=128 per pair,
                       PSUM-accumulated across the two pairs)

PSUM budget (8 banks): scores 2x[128,1024]f32 (4) + ctx 2x[128,512]f32
(2) + rotating proj/outproj/denominator bank pair (2).
"""

from collections import deque

import numpy as np
import ml_dtypes

import concourse.bass as bass
import concourse.tile as tile
from concourse import bacc, mybir
from concourse.bass_utils import run_bass_kernel_spmd

B, S, SIZE, H, D = 2, 2048, 1024, 16, 64
NCORES = 8
HGROUPS = 4                # tensor-parallel head groups
H_LOC = H // HGROUPS       # 4 heads per core
D_LOC = H_LOC * D          # 256 projection dims per core
MT = D_LOC // 128          # 2 head-pairs per core
ET = SIZE // 128           # 8 contraction tiles for projections
ST = S // 128              # 16 sequence tiles of 128
KT = S // 128              # 16 key tiles

_NC = None


def build():
    global _NC
    if _NC is not None:
        return _NC
    f32, bf16 = mybir.dt.float32, mybir.dt.bfloat16
    Exp = mybir.ActivationFunctionType.Exp

    nc = bacc.Bacc("TRN2", target_bir_lowering=False, debug=False)
    qT_d = nc.dram_tensor("qT", [SIZE, S], bf16, kind="ExternalInput").ap()
    kT_d = nc.dram_tensor("kT", [SIZE, S], bf16, kind="ExternalInput").ap()
    vT_d = nc.dram_tensor("vT", [SIZE, S], bf16, kind="ExternalInput").ap()
    WqT_d = nc.dram_tensor("WqT", [SIZE, D_LOC], bf16, kind="ExternalInput").ap()
    WkT_d = nc.dram_tensor("WkT", [SIZE, D_LOC], bf16, kind="ExternalInput").ap()
    WvT_d = nc.dram_tensor("WvT", [SIZE, D_LOC], bf16, kind="ExternalInput").ap()
    WoT_d = nc.dram_tensor("WoT", [D_LOC, SIZE], bf16, kind="ExternalInput").ap()
    bq_d = nc.dram_tensor("bq", [D_LOC], f32, kind="ExternalInput").ap()
    bk_d = nc.dram_tensor("bk", [D_LOC], f32, kind="ExternalInput").ap()
    out_d = nc.dram_tensor("out", [S, SIZE], bf16, kind="ExternalOutput").ap()

    qTt = qT_d.rearrange("(et p) s -> p et s", p=128)
    kTt = kT_d.rearrange("(et p) s -> p et s", p=128)
    vTt = vT_d.rearrange("(et p) s -> p et s", p=128)

    with tile.TileContext(nc) as tc:
        with (
            tc.tile_pool(name="persist", bufs=1) as persist,
            tc.tile_pool(name="psS", bufs=2, space="PSUM") as psS,
            tc.tile_pool(name="psC", bufs=1, space="PSUM") as psC,
            tc.tile_pool(name="psP", bufs=2, space="PSUM") as psP,
            tc.tile_pool(name="esb", bufs=5) as esb,
            tc.tile_pool(name="rden", bufs=2) as rden,
            tc.tile_pool(name="osb", bufs=3) as osb,
        ):
            # ---------- persistent SBUF: weights, inputs, activations ----------
            wk_sb = persist.tile([128, ET, D_LOC], bf16)
            wq_sb = persist.tile([128, ET, D_LOC], bf16)
            wv_sb = persist.tile([128, ET, D_LOC], bf16)
            nc.sync.dma_start(wk_sb[:], WkT_d.rearrange("(et p) m -> p et m", p=128))
            nc.sync.dma_start(wq_sb[:], WqT_d.rearrange("(et p) m -> p et m", p=128))
            bq_sb = persist.tile([128, MT], f32)
            bk_sb = persist.tile([128, MT], f32)
            nc.sync.dma_start(bq_sb[:], bq_d.rearrange("(mt p) -> p mt", p=128))
            nc.sync.dma_start(bk_sb[:], bk_d.rearrange("(mt p) -> p mt", p=128))

            # split the input stream across both HWDGE rings: qT on the sync
            # ring, kT/vT on the scalar-engine ring, so the two critical
            # tensors stream in parallel instead of FIFO on one ring
            qRows = persist.tile([128, 2, ET, S // 2], bf16)
            nc.sync.dma_start(qRows[:, 0, :, :], qTt[:, :, 0:1024])
            kRows = persist.tile([128, ET, S], bf16)
            nc.scalar.dma_start(kRows[:], kTt)
            nc.sync.dma_start(wv_sb[:], WvT_d.rearrange("(et p) m -> p et m", p=128))
            vRows = persist.tile([128, ET, S], bf16)
            nc.scalar.dma_start(vRows[:], vTt)
            nc.sync.dma_start(qRows[:, 1, :, :], qTt[:, :, 1024:2048])
            wo_sb = persist.tile([128, MT, SIZE], bf16)
            nc.sync.dma_start(wo_sb[:], WoT_d.rearrange("(hp p) o -> p hp o", p=128))

            qh_sb = persist.tile([128, MT, S], bf16)   # [dim within pair, pair, s]
            kh_sb = persist.tile([128, MT, S], bf16)
            vh_sb = persist.tile([128, H_LOC, ST, D], bf16)  # [s%128, head, s//128, d]
            ctx_sb = persist.tile([128, MT, S], bf16)  # normalized ctxT
            eacc = [persist.tile([128, 2048], bf16, name=f"eacc{a}")
                    for a in range(4)]
            ones_f = persist.tile([128, 64], f32)
            nc.vector.memset(ones_f[:], 1.0)
            ones_b = persist.tile([128, 64], bf16)
            nc.vector.tensor_copy(ones_b[:], ones_f[:])
            warm = persist.tile([128, 1], f32)
            # pre-load the ACT exp table set during the head phase
            nc.scalar.activation(warm[:], ones_f[:, 0:1], Exp)

            # ---------- interleave unit machinery ----------
            units = deque()

            def drain_units(n):
                for _ in range(min(n, len(units))):
                    units.popleft()()

            def qk_chunk_units(rows_ap_fn, wsb, pr, dst, bsb, chunk):
                # one 512-wide s-chunk of a Q/K projection for pair pr:
                # 8 et-accumulated matmuls + bias-add evacuation
                st_ = {}

                def mk(et0):
                    def f():
                        if et0 == 0:
                            st_['p'] = psP.tile([128, 512], f32, tag="p", name="pqk")
                        for et in range(et0, et0 + 4):
                            nc.tensor.matmul(
                                st_['p'][:],
                                wsb[:, et, pr * 128:(pr + 1) * 128],
                                rows_ap_fn(et, chunk),
                                start=(et == 0), stop=(et == ET - 1))
                    return f

                def ev():
                    nc.vector.tensor_scalar_add(
                        dst[:, pr, chunk * 512:(chunk + 1) * 512],
                        st_['p'][:], bsb[:, pr:pr + 1])
                return [mk(0), mk(4), ev]

            def k_rows(et, chunk):
                return kRows[:, et, chunk * 512:(chunk + 1) * 512]

            def q_rows(et, chunk):
                return qRows[:, chunk // 2, et, (chunk % 2) * 512:
                             (chunk % 2 + 1) * 512]

            def v_fill_units(pr, b):
                # V projection for pair pr, s-tiles 4b..4b+3 (one PSUM bank).
                # Each s-tile's 8-et accumulation group runs to completion
                # before the next starts: start=True clears has_written for
                # the whole bank on the written partitions, so column-split
                # groups on the same partitions must not interleave.
                st_ = {}

                def mk(i):
                    def f():
                        if i == 0:
                            st_['p'] = psP.tile([128, 512], f32, tag="p", name="pv")
                        st = 4 * b + i
                        for et in range(ET):
                            nc.tensor.matmul(
                                st_['p'][:, i * 128:(i + 1) * 128],
                                vRows[:, et, st * 128:(st + 1) * 128],
                                wv_sb[:, et, pr * 128:(pr + 1) * 128],
                                start=(et == 0), stop=(et == ET - 1))
                    return f

                def ev():
                    for i in range(4):
                        nc.vector.tensor_copy(
                            vh_sb[:, 2 * pr:2 * pr + 2, 4 * b + i, :],
                            st_['p'][:, i * 128:(i + 1) * 128]
                            .rearrange("p (h d) -> p h d", h=2))
                return [mk(0), mk(1), mk(2), mk(3), ev]

            def outproj_units(st, on_scalar):
                # output projection for s-tile st: two [128,512] fills
                # (c=256 via PSUM accumulation over the 2 pairs), evacuated
                # to one bf16 row tile and DMA'd out
                st_ = {}

                def mm(ot):
                    def f():
                        if ot == 0:
                            st_['o'] = osb.tile([128, 1024], bf16, tag="o",
                                                name="orow")
                        st_[ot] = psP.tile([128, 512], f32, tag="p", name="po")
                        for hp in range(MT):
                            nc.tensor.matmul(
                                st_[ot][:],
                                ctx_sb[:, hp, st * 128:(st + 1) * 128],
                                wo_sb[:, hp, ot * 512:(ot + 1) * 512],
                                start=(hp == 0), stop=(hp == MT - 1))
                    return f

                def ev(ot):
                    def f():
                        dst = st_['o'][:, ot * 512:(ot + 1) * 512]
                        if on_scalar:
                            nc.scalar.copy(dst, st_[ot][:])
                        else:
                            nc.vector.tensor_copy(dst, st_[ot][:])
                        if ot == 1:
                            nc.sync.dma_start(
                                out_d[st * 128:(st + 1) * 128, :], st_['o'][:])
                    return f
                return [mm(0), ev(0), mm(1), ev(1)]

            def sweep_tail_units(cj, pr, q0):
                # softmax denominators for the finished sweep: reduce over
                # partitions and broadcast in one shot (ones[128,64] lhsT,
                # accumulated over the 4 eacc tiles), then reciprocal and
                # normalize straight out of the ctx PSUM banks
                st_ = {}
                out_units = []
                for j in range(2):
                    def mmj(j=j):
                        den = psP.tile([128, 512], f32, tag="p", name="den")
                        st_[j] = den
                        for a in range(4):
                            for hi in range(2):
                                nc.tensor.matmul(
                                    den[hi * D:(hi + 1) * D, :],
                                    ones_b[:],
                                    eacc[a][:, hi * 1024 + j * 512:
                                            hi * 1024 + (j + 1) * 512],
                                    start=(a == 0), stop=(a == 3))

                    def nrm(j=j):
                        rd = rden.tile([128, 512], f32, tag="r", name="rd")
                        nc.vector.reciprocal(rd[:], st_[j][:])
                        nc.vector.tensor_mul(
                            ctx_sb[:, pr, q0 + j * 512:q0 + (j + 1) * 512],
                            cj[j][:], rd[:])
                    out_units.extend([mmj, nrm])
                return out_units

            # ---------- head phase: minimal critical path (DMA-shadowed) -------
            for c in range(2):
                for u in qk_chunk_units(q_rows, wq_sb, 0, qh_sb, bq_sb, c):
                    u()
            for c in range(2):
                for u in qk_chunk_units(k_rows, wk_sb, 0, kh_sb, bk_sb, c):
                    u()

            # ---------- sweeps: ACT-paced attention + interleaved units ----------
            pending_tail = None
            for si, (pr, sw) in enumerate([(0, 0), (0, 1), (1, 0), (1, 1)]):
                q0 = sw * 1024
                if si == 0:
                    units.extend(v_fill_units(0, 0))
                    units.extend(qk_chunk_units(k_rows, wk_sb, 0,
                                                kh_sb, bk_sb, 2))
                    units.extend(v_fill_units(0, 1))
                    units.extend(qk_chunk_units(k_rows, wk_sb, 0,
                                                kh_sb, bk_sb, 3))
                    for c in (2, 3):
                        units.extend(qk_chunk_units(q_rows, wq_sb, 0,
                                                    qh_sb, bq_sb, c))
                    for b in (2, 3):
                        units.extend(v_fill_units(0, b))
                    units.extend(qk_chunk_units(k_rows, wk_sb, 1,
                                                kh_sb, bk_sb, 0))
                elif si == 1:
                    for c in (1, 2):
                        units.extend(qk_chunk_units(k_rows, wk_sb, 1,
                                                    kh_sb, bk_sb, c))
                    for c in (0, 1):
                        units.extend(qk_chunk_units(q_rows, wq_sb, 1,
                                                    qh_sb, bq_sb, c))
                    for b in range(4):
                        units.extend(v_fill_units(1, b))
                elif si == 2:
                    units.extend(qk_chunk_units(k_rows, wk_sb, 1,
                                                kh_sb, bk_sb, 3))
                    for c in (2, 3):
                        units.extend(qk_chunk_units(q_rows, wq_sb, 1,
                                                    qh_sb, bq_sb, c))
                else:
                    for st in range(6):
                        units.extend(outproj_units(st, on_scalar=False))

                cj = [psC.tile([128, 512], f32, tag=f"c{j}", name=f"c{j}")
                      for j in range(2)]

                def emit_ctx(kt, e_t, cj=cj, pr=pr):
                    for j in range(2):
                        for hi in range(2):
                            nc.tensor.matmul(
                                cj[j][hi * D:(hi + 1) * D, :],
                                vh_sb[:, 2 * pr + hi, kt, :],
                                e_t[:, hi * 1024 + j * 512:
                                    hi * 1024 + (j + 1) * 512],
                                start=(kt == 0), stop=(kt == KT - 1))

                lagged = deque()
                for kt in range(KT):
                    scs = []
                    for hi in range(2):
                        sc = psS.tile([128, 1024], f32, tag="sc", name="sc")
                        po = hi * D
                        for j in range(2):
                            nc.tensor.matmul(
                                sc[:, j * 512:(j + 1) * 512],
                                kh_sb[po:po + D, pr, kt * 128:(kt + 1) * 128],
                                qh_sb[po:po + D, pr, q0 + j * 512:
                                      q0 + (j + 1) * 512],
                                start=True, stop=True)
                        scs.append(sc)
                    if kt == 0 and pending_tail is not None:
                        # previous sweep's denominator/normalize: must be
                        # emitted before this sweep's first eacc overwrite
                        for u in pending_tail:
                            u()
                        pending_tail = None
                    drain_units(1)
                    e_t = esb.tile([128, 2048], bf16, tag="e", name="e")
                    for hi in range(2):
                        nc.scalar.activation(
                            e_t[:, hi * 1024:(hi + 1) * 1024], scs[hi][:], Exp)
                    # ctx lags one kt so the PE never FIFO-blocks on ACT;
                    # sweep 0 lags deeper so late-arriving V tiles (behind
                    # kT on the DMA ring) can't stall the score stream
                    lagged.append((kt, e_t))
                    if len(lagged) > (4 if si == 0 else 1):
                        emit_ctx(*lagged.popleft())
                    a = kt % 4
                    if kt < 4:
                        nc.vector.tensor_copy(eacc[a][:], e_t[:])
                    else:
                        nc.vector.tensor_add(eacc[a][:], eacc[a][:], e_t[:])
                    # sweep 0 carries the largest unit backlog: drain faster
                    # so every cross-sweep writer lands inside this sweep
                    drain_units(2 if si == 0 else 1)
                while lagged:
                    emit_ctx(*lagged.popleft())
                pending_tail = sweep_tail_units(cj, pr, q0)

            # ---------- tail: last denominators, leftovers, second out half ----
            # wide fills reuse the now-idle scores banks (deeper rotation than
            # the "p" pair), one evacuation per s-tile alternating ACT/DVE,
            # and each 512-q chunk's tiles start right after its normalize
            drain_units(len(units))

            def tail_outproj(st, on_scalar):
                o_t = osb.tile([128, 1024], bf16, tag="o", name="orow")
                pt = psS.tile([128, 1024], f32, tag="sc", name="pot")
                for ot in range(2):
                    for hp in range(MT):
                        nc.tensor.matmul(
                            pt[:, ot * 512:(ot + 1) * 512],
                            ctx_sb[:, hp, st * 128:(st + 1) * 128],
                            wo_sb[:, hp, ot * 512:(ot + 1) * 512],
                            start=(hp == 0), stop=(hp == MT - 1))
                if on_scalar:
                    nc.scalar.copy(o_t[:], pt[:])
                else:
                    nc.vector.tensor_copy(o_t[:], pt[:])
                nc.sync.dma_start(out_d[st * 128:(st + 1) * 128, :], o_t[:])

            pending_tail[0]()          # den j=0
            pending_tail[1]()          # recip+normalize j=0 (DVE queue empty)
            # st6/7 need no last-sweep normalize: their PE fills and ACT
            # evacuations run concurrently with the j=0 reciprocal on DVE
            tail_outproj(6, True)
            tail_outproj(7, True)
            pending_tail[2]()          # den j=1
            pending_tail[3]()          # recip+normalize j=1
            for i, st in enumerate((8, 9, 10, 11, 12, 13, 14, 15)):
                tail_outproj(st, i % 2 == 0)

    nc.compile()
    _NC = nc
    return nc


def prepare_in_maps(inputs):
    q, k, v = inputs["q"], inputs["k"], inputs["v"]
    Wq, bq = inputs["Wq"], inputs["bq"]
    Wk, bk = inputs["Wk"], inputs["bk"]
    Wv = inputs["Wv"]
    Wo = inputs["Wo"]
    sc = np.float32(1.0 / np.sqrt(D))

    f32, bf = np.float32, ml_dtypes.bfloat16
    qT = [q[b].T.astype(bf) for b in range(B)]
    kT = [k[b].T.astype(bf) for b in range(B)]
    vT = [v[b].T.astype(bf) for b in range(B)]
    WqTs = (Wq.T * sc).astype(bf)   # scale folded into Wq
    WkT = Wk.T.astype(bf)
    WvT = Wv.T.astype(bf)
    WoT = Wo.T.astype(bf)           # [c, o]
    bqs = (bq * sc).astype(f32)

    in_maps = []
    for core in range(NCORES):
        b, hg = divmod(core, HGROUPS)
        sl = slice(hg * D_LOC, (hg + 1) * D_LOC)
        in_maps.append({
            "qT": qT[b], "kT": kT[b], "vT": vT[b],
            "WqT": np.ascontiguousarray(WqTs[:, sl]),
            "WkT": np.ascontiguousarray(WkT[:, sl]),
            "WvT": np.ascontiguousarray(WvT[:, sl]),
            "WoT": np.ascontiguousarray(WoT[sl, :]),
            "bq": np.ascontiguousarray(bqs[sl]),
            "bk": np.ascontiguousarray(bk[sl].astype(f32)),
        })
    return in_maps


def gather(results, inputs):
    # host epilogue: sum the 4 tensor-parallel partials per batch and add the
    # constant row bv @ Wo.T + bo (the value bias commutes through softmax)
    const = (inputs["bv"].astype(np.float64) @ inputs["Wo"].astype(np.float64).T
             + inputs["bo"].astype(np.float64)).astype(np.float32)
    full = np.empty((B, S, SIZE), np.float32)
    for b in range(B):
        acc = results[b * HGROUPS]["out"].astype(np.float32)
        for hg in range(1, HGROUPS):
            acc += results[b * HGROUPS + hg]["out"].astype(np.float32)
        full[b] = acc + const[None, :]
    return full


def kernel(**inputs):
    nc = build()
    in_maps = prepare_in_maps(inputs)
    res = run_bass_kernel_spmd(nc, in_maps, core_ids=list(range(NCORES)), trace=False)
    return gather(res.results, inputs)


# revision 37
# speedup vs baseline: 1.0119x; 1.0119x over previous
"""Fused multi-head attention forward (B=2, S=2048, SIZE=1024, H=16) on 8
Trainium2 NeuronCores.

Sharding: 2-way data parallel over batch x 4-way tensor parallel over heads
(Megatron style). Each core computes 4 heads of one batch element end-to-end
(QKV projections for its 256-dim slice, attention, and a partial output
projection); the host sums the 4 partials per batch and adds the output
bias. The value-projection bias drops out of attention algebraically
(softmax rows sum to 1), so the host folds `bv @ Wo.T` into that same
constant row.

Single software-pipelined schedule built around the scalar-engine exp
stream (the hard floor: S*S*H_LOC exps/core at 1 elem/lane/cycle). The
attention kt-loop keeps ACT busy with [128,1024] exp instructions while
the PE fills its idle cycles from a unit queue carrying the QKV
projections for the *next* head pair, the output projection for
*previous* q-ranges, and the softmax-denominator reduction:

  qhT/khT [dim, s]  <- qT/kT rows as rhs, WqT/WkT as lhsT (no transposes)
  vh      [s, dim]  <- vT rows as lhsT, WvT as rhs
  scoresT [k, q]    <- khT as lhsT (c=64); two heads packed in PE row
                       groups 0-63 / 64-127 run as concurrent streams
  exp     ACT PSUM->SBUF bf16, one [128,1024] instruction per head
  ctxT    [dim, q]  <- vh as lhsT, exp(scoresT) as rhs; two heads packed
                       in PE col groups (output partitions 0-63 / 64-127)
  denom:  exp tiles accumulated on DVE into 4 interleaved bf16 tiles,
          then reduced over partitions AND broadcast in one shot by
          ones[128,64] matmuls accumulated in PSUM; reciprocal +
          normalize on DVE read the ctx PSUM banks directly
  out     [s, o]    <- ctxT as lhsT, WoT as rhs (c=128 per pair,
                       PSUM-accumulated across the two pairs)

PSUM budget (8 banks): scores 2x[128,1024]f32 (4) + ctx 2x[128,512]f32
(2) + rotating proj/outproj/denominator bank pair (2).
"""

from collections import deque

import numpy as np
import ml_dtypes

import concourse.bass as bass
import concourse.tile as tile
from concourse import bacc, mybir
from concourse.bass_utils import run_bass_kernel_spmd

B, S, SIZE, H, D = 2, 2048, 1024, 16, 64
NCORES = 8
HGROUPS = 4                # tensor-parallel head groups
H_LOC = H // HGROUPS       # 4 heads per core
D_LOC = H_LOC * D          # 256 projection dims per core
MT = D_LOC // 128          # 2 head-pairs per core
ET = SIZE // 128           # 8 contraction tiles for projections
ST = S // 128              # 16 sequence tiles of 128
KT = S // 128              # 16 key tiles

_NC = None


def build():
    global _NC
    if _NC is not None:
        return _NC
    f32, bf16 = mybir.dt.float32, mybir.dt.bfloat16
    Exp = mybir.ActivationFunctionType.Exp

    nc = bacc.Bacc("TRN2", target_bir_lowering=False, debug=False)
    qT_d = nc.dram_tensor("qT", [SIZE, S], bf16, kind="ExternalInput").ap()
    kT_d = nc.dram_tensor("kT", [SIZE, S], bf16, kind="ExternalInput").ap()
    vT_d = nc.dram_tensor("vT", [SIZE, S], bf16, kind="ExternalInput").ap()
    WqT_d = nc.dram_tensor("WqT", [SIZE, D_LOC], bf16, kind="ExternalInput").ap()
    WkT_d = nc.dram_tensor("WkT", [SIZE, D_LOC], bf16, kind="ExternalInput").ap()
    WvT_d = nc.dram_tensor("WvT", [SIZE, D_LOC], bf16, kind="ExternalInput").ap()
    WoT_d = nc.dram_tensor("WoT", [D_LOC, SIZE], bf16, kind="ExternalInput").ap()
    bq_d = nc.dram_tensor("bq", [D_LOC], f32, kind="ExternalInput").ap()
    bk_d = nc.dram_tensor("bk", [D_LOC], f32, kind="ExternalInput").ap()
    out_d = nc.dram_tensor("out", [S, SIZE], bf16, kind="ExternalOutput").ap()

    qTt = qT_d.rearrange("(et p) s -> p et s", p=128)
    kTt = kT_d.rearrange("(et p) s -> p et s", p=128)
    vTt = vT_d.rearrange("(et p) s -> p et s", p=128)

    with tile.TileContext(nc) as tc:
        with (
            tc.tile_pool(name="persist", bufs=1) as persist,
            tc.tile_pool(name="psS", bufs=2, space="PSUM") as psS,
            tc.tile_pool(name="psC", bufs=1, space="PSUM") as psC,
            tc.tile_pool(name="psP", bufs=2, space="PSUM") as psP,
            tc.tile_pool(name="esb", bufs=5) as esb,
            tc.tile_pool(name="rden", bufs=2) as rden,
            tc.tile_pool(name="osb", bufs=3) as osb,
        ):
            # ---------- persistent SBUF: weights, inputs, activations ----------
            wk_sb = persist.tile([128, ET, D_LOC], bf16)
            wq_sb = persist.tile([128, ET, D_LOC], bf16)
            wv_sb = persist.tile([128, ET, D_LOC], bf16)
            nc.sync.dma_start(wk_sb[:], WkT_d.rearrange("(et p) m -> p et m", p=128))
            nc.sync.dma_start(wq_sb[:], WqT_d.rearrange("(et p) m -> p et m", p=128))
            bq_sb = persist.tile([128, MT], f32)
            bk_sb = persist.tile([128, MT], f32)
            nc.sync.dma_start(bq_sb[:], bq_d.rearrange("(mt p) -> p mt", p=128))
            nc.sync.dma_start(bk_sb[:], bk_d.rearrange("(mt p) -> p mt", p=128))

            # split the input stream across both HWDGE rings: qT on the sync
            # ring, kT/vT on the scalar-engine ring, so the two critical
            # tensors stream in parallel instead of FIFO on one ring
            qRows = persist.tile([128, 2, ET, S // 2], bf16)
            nc.sync.dma_start(qRows[:, 0, :, :], qTt[:, :, 0:1024])
            kRows = persist.tile([128, ET, S], bf16)
            nc.scalar.dma_start(kRows[:], kTt)
            nc.sync.dma_start(wv_sb[:], WvT_d.rearrange("(et p) m -> p et m", p=128))
            vRows = persist.tile([128, ET, S], bf16)
            nc.scalar.dma_start(vRows[:], vTt)
            nc.sync.dma_start(qRows[:, 1, :, :], qTt[:, :, 1024:2048])
            wo_sb = persist.tile([128, MT, SIZE], bf16)
            nc.sync.dma_start(wo_sb[:], WoT_d.rearrange("(hp p) o -> p hp o", p=128))

            qh_sb = persist.tile([128, MT, S], bf16)   # [dim within pair, pair, s]
            kh_sb = persist.tile([128, MT, S], bf16)
            vh_sb = persist.tile([128, H_LOC, ST, D], bf16)  # [s%128, head, s//128, d]
            ctx_sb = persist.tile([128, MT, S], bf16)  # normalized ctxT
            eacc = [persist.tile([128, 2048], bf16, name=f"eacc{a}")
                    for a in range(4)]
            ones_f = persist.tile([128, 64], f32)
            nc.vector.memset(ones_f[:], 1.0)
            ones_b = persist.tile([128, 64], bf16)
            nc.vector.tensor_copy(ones_b[:], ones_f[:])
            warm = persist.tile([128, 1], f32)
            # pre-load the ACT exp table set during the head phase
            nc.scalar.activation(warm[:], ones_f[:, 0:1], Exp)

            # ---------- interleave unit machinery ----------
            units = deque()

            def drain_units(n):
                for _ in range(min(n, len(units))):
                    units.popleft()()

            def qk_chunk_units(rows_ap_fn, wsb, pr, dst, bsb, chunk):
                # one 512-wide s-chunk of a Q/K projection for pair pr:
                # 8 et-accumulated matmuls + bias-add evacuation
                st_ = {}

                def mk(et0):
                    def f():
                        if et0 == 0:
                            st_['p'] = psP.tile([128, 512], f32, tag="p", name="pqk")
                        for et in range(et0, et0 + 4):
                            nc.tensor.matmul(
                                st_['p'][:],
                                wsb[:, et, pr * 128:(pr + 1) * 128],
                                rows_ap_fn(et, chunk),
                                start=(et == 0), stop=(et == ET - 1))
                    return f

                def ev():
                    nc.vector.tensor_scalar_add(
                        dst[:, pr, chunk * 512:(chunk + 1) * 512],
                        st_['p'][:], bsb[:, pr:pr + 1])
                return [mk(0), mk(4), ev]

            def k_rows(et, chunk):
                return kRows[:, et, chunk * 512:(chunk + 1) * 512]

            def q_rows(et, chunk):
                return qRows[:, chunk // 2, et, (chunk % 2) * 512:
                             (chunk % 2 + 1) * 512]

            def v_fill_units(pr, b):
                # V projection for pair pr, s-tiles 4b..4b+3 (one PSUM bank).
                # Each s-tile's 8-et accumulation group runs to completion
                # before the next starts: start=True clears has_written for
                # the whole bank on the written partitions, so column-split
                # groups on the same partitions must not interleave.
                st_ = {}

                def mk(i):
                    def f():
                        if i == 0:
                            st_['p'] = psP.tile([128, 512], f32, tag="p", name="pv")
                        st = 4 * b + i
                        for et in range(ET):
                            nc.tensor.matmul(
                                st_['p'][:, i * 128:(i + 1) * 128],
                                vRows[:, et, st * 128:(st + 1) * 128],
                                wv_sb[:, et, pr * 128:(pr + 1) * 128],
                                start=(et == 0), stop=(et == ET - 1))
                    return f

                def ev():
                    for i in range(4):
                        nc.vector.tensor_copy(
                            vh_sb[:, 2 * pr:2 * pr + 2, 4 * b + i, :],
                            st_['p'][:, i * 128:(i + 1) * 128]
                            .rearrange("p (h d) -> p h d", h=2))
                return [mk(0), mk(1), mk(2), mk(3), ev]

            def outproj_units(st, on_scalar):
                # output projection for s-tile st: two [128,512] fills
                # (c=256 via PSUM accumulation over the 2 pairs), evacuated
                # to one bf16 row tile and DMA'd out
                st_ = {}

                def mm(ot):
                    def f():
                        if ot == 0:
                            st_['o'] = osb.tile([128, 1024], bf16, tag="o",
                                                name="orow")
                        st_[ot] = psP.tile([128, 512], f32, tag="p", name="po")
                        for hp in range(MT):
                            nc.tensor.matmul(
                                st_[ot][:],
                                ctx_sb[:, hp, st * 128:(st + 1) * 128],
                                wo_sb[:, hp, ot * 512:(ot + 1) * 512],
                                start=(hp == 0), stop=(hp == MT - 1))
                    return f

                def ev(ot):
                    def f():
                        dst = st_['o'][:, ot * 512:(ot + 1) * 512]
                        if on_scalar:
                            nc.scalar.copy(dst, st_[ot][:])
                        else:
                            nc.vector.tensor_copy(dst, st_[ot][:])
                        if ot == 1:
                            nc.sync.dma_start(
                                out_d[st * 128:(st + 1) * 128, :], st_['o'][:])
                    return f
                return [mm(0), ev(0), mm(1), ev(1)]

            def sweep_tail_units(cj, pr, q0):
                # softmax denominators for the finished sweep: reduce over
                # partitions and broadcast in one shot (ones[128,64] lhsT,
                # accumulated over the 4 eacc tiles), then reciprocal and
                # normalize straight out of the ctx PSUM banks
                st_ = {}
                out_units = []
                for j in range(2):
                    def mmj(j=j):
                        den = psP.tile([128, 512], f32, tag="p", name="den")
                        st_[j] = den
                        for a in range(4):
                            for hi in range(2):
                                nc.tensor.matmul(
                                    den[hi * D:(hi + 1) * D, :],
                                    ones_b[:],
                                    eacc[a][:, hi * 1024 + j * 512:
                                            hi * 1024 + (j + 1) * 512],
                                    start=(a == 0), stop=(a == 3))

                    def nrm(j=j):
                        rd = rden.tile([128, 512], f32, tag="r", name="rd")
                        nc.vector.reciprocal(rd[:], st_[j][:])
                        nc.vector.tensor_mul(
                            ctx_sb[:, pr, q0 + j * 512:q0 + (j + 1) * 512],
                            cj[j][:], rd[:])
                    out_units.extend([mmj, nrm])
                return out_units

            # ---------- head phase: minimal critical path (DMA-shadowed) -------
            for c in range(2):
                for u in qk_chunk_units(q_rows, wq_sb, 0, qh_sb, bq_sb, c):
                    u()
            for c in range(2):
                for u in qk_chunk_units(k_rows, wk_sb, 0, kh_sb, bk_sb, c):
                    u()

            # ---------- sweeps: ACT-paced attention + interleaved units ----------
            pending_tail = None
            for si, (pr, sw) in enumerate([(0, 0), (0, 1), (1, 0), (1, 1)]):
                q0 = sw * 1024
                if si == 0:
                    units.extend(v_fill_units(0, 0))
                    units.extend(qk_chunk_units(k_rows, wk_sb, 0,
                                                kh_sb, bk_sb, 2))
                    units.extend(v_fill_units(0, 1))
                    units.extend(qk_chunk_units(k_rows, wk_sb, 0,
                                                kh_sb, bk_sb, 3))
                    for c in (2, 3):
                        units.extend(qk_chunk_units(q_rows, wq_sb, 0,
                                                    qh_sb, bq_sb, c))
                    for b in (2, 3):
                        units.extend(v_fill_units(0, b))
                    units.extend(qk_chunk_units(k_rows, wk_sb, 1,
                                                kh_sb, bk_sb, 0))
                elif si == 1:
                    for c in (1, 2):
                        units.extend(qk_chunk_units(k_rows, wk_sb, 1,
                                                    kh_sb, bk_sb, c))
                    for c in (0, 1):
                        units.extend(qk_chunk_units(q_rows, wq_sb, 1,
                                                    qh_sb, bq_sb, c))
                    for b in range(4):
                        units.extend(v_fill_units(1, b))
                elif si == 2:
                    units.extend(qk_chunk_units(k_rows, wk_sb, 1,
                                                kh_sb, bk_sb, 3))
                    for c in (2, 3):
                        units.extend(qk_chunk_units(q_rows, wq_sb, 1,
                                                    qh_sb, bq_sb, c))
                else:
                    for st in range(8):
                        units.extend(outproj_units(st, on_scalar=False))

                cj = [psC.tile([128, 512], f32, tag=f"c{j}", name=f"c{j}")
                      for j in range(2)]

                def emit_ctx(kt, e_t, cj=cj, pr=pr):
                    for j in range(2):
                        for hi in range(2):
                            nc.tensor.matmul(
                                cj[j][hi * D:(hi + 1) * D, :],
                                vh_sb[:, 2 * pr + hi, kt, :],
                                e_t[:, hi * 1024 + j * 512:
                                    hi * 1024 + (j + 1) * 512],
                                start=(kt == 0), stop=(kt == KT - 1))

                lagged = deque()
                for kt in range(KT):
                    scs = []
                    for hi in range(2):
                        sc = psS.tile([128, 1024], f32, tag="sc", name="sc")
                        po = hi * D
                        for j in range(2):
                            nc.tensor.matmul(
                                sc[:, j * 512:(j + 1) * 512],
                                kh_sb[po:po + D, pr, kt * 128:(kt + 1) * 128],
                                qh_sb[po:po + D, pr, q0 + j * 512:
                                      q0 + (j + 1) * 512],
                                start=True, stop=True)
                        scs.append(sc)
                    if kt == 0 and pending_tail is not None:
                        # previous sweep's denominator/normalize: must be
                        # emitted before this sweep's first eacc overwrite
                        for u in pending_tail:
                            u()
                        pending_tail = None
                    drain_units(1)
                    e_t = esb.tile([128, 2048], bf16, tag="e", name="e")
                    for hi in range(2):
                        nc.scalar.activation(
                            e_t[:, hi * 1024:(hi + 1) * 1024], scs[hi][:], Exp)
                    # ctx lags one kt so the PE never FIFO-blocks on ACT;
                    # sweep 0 lags deeper so late-arriving V tiles (behind
                    # kT on the DMA ring) can't stall the score stream
                    lagged.append((kt, e_t))
                    if len(lagged) > (4 if si == 0 else 1):
                        emit_ctx(*lagged.popleft())
                    a = kt % 4
                    if kt < 4:
                        nc.vector.tensor_copy(eacc[a][:], e_t[:])
                    else:
                        nc.vector.tensor_add(eacc[a][:], eacc[a][:], e_t[:])
                    # sweep 0 carries the largest unit backlog: drain faster
                    # so every cross-sweep writer lands inside this sweep
                    drain_units(2 if si == 0 else 1)
                while lagged:
                    emit_ctx(*lagged.popleft())
                pending_tail = sweep_tail_units(cj, pr, q0)

            # ---------- tail: last denominators, leftovers, second out half ----
            # wide fills reuse the now-idle scores banks (deeper rotation than
            # the "p" pair), one evacuation per s-tile alternating ACT/DVE,
            # and each 512-q chunk's tiles start right after its normalize
            drain_units(len(units))

            def tail_outproj(st):
                o_t = osb.tile([128, 1024], bf16, tag="o", name="orow")
                pt = psS.tile([128, 1024], f32, tag="sc", name="pot")
                for ot in range(2):
                    for hp in range(MT):
                        nc.tensor.matmul(
                            pt[:, ot * 512:(ot + 1) * 512],
                            ctx_sb[:, hp, st * 128:(st + 1) * 128],
                            wo_sb[:, hp, ot * 512:(ot + 1) * 512],
                            start=(hp == 0), stop=(hp == MT - 1))
                if st % 2 == 0:
                    nc.scalar.copy(o_t[:], pt[:])
                else:
                    nc.vector.tensor_copy(o_t[:], pt[:])
                nc.sync.dma_start(out_d[st * 128:(st + 1) * 128, :], o_t[:])

            pending_tail[0]()          # den j=0
            pending_tail[1]()          # recip+normalize j=0
            for st in (8, 9, 10):
                tail_outproj(st)
            pending_tail[2]()          # den j=1
            pending_tail[3]()          # recip+normalize j=1
            for st in (11, 12, 13, 14, 15):
                tail_outproj(st)

    nc.compile()
    _NC = nc
    return nc


def prepare_in_maps(inputs):
    q, k, v = inputs["q"], inputs["k"], inputs["v"]
    Wq, bq = inputs["Wq"], inputs["bq"]
    Wk, bk = inputs["Wk"], inputs["bk"]
    Wv = inputs["Wv"]
    Wo = inputs["Wo"]
    sc = np.float32(1.0 / np.sqrt(D))

    f32, bf = np.float32, ml_dtypes.bfloat16
    qT = [q[b].T.astype(bf) for b in range(B)]
    kT = [k[b].T.astype(bf) for b in range(B)]
    vT = [v[b].T.astype(bf) for b in range(B)]
    WqTs = (Wq.T * sc).astype(bf)   # scale folded into Wq
    WkT = Wk.T.astype(bf)
    WvT = Wv.T.astype(bf)
    WoT = Wo.T.astype(bf)           # [c, o]
    bqs = (bq * sc).astype(f32)

    in_maps = []
    for core in range(NCORES):
        b, hg = divmod(core, HGROUPS)
        sl = slice(hg * D_LOC, (hg + 1) * D_LOC)
        in_maps.append({
            "qT": qT[b], "kT": kT[b], "vT": vT[b],
            "WqT": np.ascontiguousarray(WqTs[:, sl]),
            "WkT": np.ascontiguousarray(WkT[:, sl]),
            "WvT": np.ascontiguousarray(WvT[:, sl]),
            "WoT": np.ascontiguousarray(WoT[sl, :]),
            "bq": np.ascontiguousarray(bqs[sl]),
            "bk": np.ascontiguousarray(bk[sl].astype(f32)),
        })
    return in_maps


def gather(results, inputs):
    # host epilogue: sum the 4 tensor-parallel partials per batch and add the
    # constant row bv @ Wo.T + bo (the value bias commutes through softmax)
    const = (inputs["bv"].astype(np.float64) @ inputs["Wo"].astype(np.float64).T
             + inputs["bo"].astype(np.float64)).astype(np.float32)
    full = np.empty((B, S, SIZE), np.float32)
    for b in range(B):
        acc = results[b * HGROUPS]["out"].astype(np.float32)
        for hg in range(1, HGROUPS):
            acc += results[b * HGROUPS + hg]["out"].astype(np.float32)
        full[b] = acc + const[None, :]
    return full


def kernel(**inputs):
    nc = build()
    in_maps = prepare_in_maps(inputs)
    res = run_bass_kernel_spmd(nc, in_maps, core_ids=list(range(NCORES)), trace=False)
    return gather(res.results, inputs)
